# revision 1
# baseline (speedup 1.0000x reference)
"""Trainium2 Bass kernel for nn_NodeModel (GNN message passing + 3-layer node MLP).

Strategy (node-parallel, 8 cores):
  - Host: sort edges by destination node, bucket them into 128-node tiles,
    pad each tile's edge list to K_CH chunks of 128 edges. Nodes are sharded
    contiguously across the 8 cores (12544 padded nodes each).
  - Device (per core, per 128-node tile):
      aggT[h, n] = sum_k edge_chunk_k[e, h].T @ onehot(col_local_k)[e, n]
      (one-hot built on DVE via iota==col compare; matmul accumulates in PSUM)
      then fused 3-layer MLP with LayerNorm + shifted-softplus, activations
      kept transposed [h, node]; LN stats computed after a PE transpose to
      [node, h]; scale/shift+softplus fused into one ACT op in [h, node].
  - -log(2) of ssp folded into the next layer's bias (host-precomputed);
    final layer subtracts it explicitly.
"""

import os
import sys

import numpy as np

sys.path.insert(0, "/opt/trn_rl_repo")

import bass_rust as _bass_rust
import ml_dtypes

from concourse import bacc, bass, hw_specs, mybir
from concourse import tile as tile_mod
from concourse.bass_utils import run_bass_kernel_spmd
from concourse.masks import make_identity


class _Bacc(bacc.Bacc):
    """Bacc with the ACT table chooser pinned to the single function set
    that holds Ln+Exp+Copy+Identity. The default greedy chooser alternates
    between per-func sets, costing a ~1.3us ACT_TABLE_LOAD per switch."""

    def insert_act_table_loads(self):
        has_activation = any(
            isinstance(i, mybir.InstActivation)
            for b in self.main_func.blocks
            for i in b.instructions
        )
        if not has_activation:
            return
        keep = "natural_log_exp_and_others"
        tables = [
            (n, (s if n == keep else set()))
            for n, s in hw_specs.get_activation_tables(self.m.arch).items()
        ]
        _bass_rust.insert_act_table_loads(self, tables)


LOG2 = float(np.log(2.0))
N, E, H = 100000, 600000, 128
NC = 8
P = 128
TPC = 98                 # 128-node tiles per core
NPC = TPC * P            # nodes per core (12544)
NPAD = NPC * NC          # padded node count (100352)
NT = NPAD // P           # total node tiles (784)

F32 = mybir.dt.float32
F32R = mybir.dt.float32r
BF16 = mybir.dt.bfloat16

LAST_RESULT = None  # BassKernelResults of the most recent run (for profiling)


def _host_prep(x, edge_index, edge_attr):
    col = np.asarray(edge_index)[1].astype(np.int64)
    ea = np.ascontiguousarray(np.asarray(edge_attr, dtype=np.float32))
    order = np.argsort(col, kind="stable")
    col_s = col[order]
    tile_of = col_s >> 7
    counts = np.bincount(tile_of, minlength=NT)
    K = int(np.ceil(counts.max() / P))
    S = K * P
    starts = np.zeros(NT + 1, np.int64)
    starts[1:] = np.cumsum(counts)
    pos = np.arange(E) - starts[tile_of]
    slot = tile_of * S + pos
    slot_edge = np.zeros(NT * S, np.int64)
    slot_edge[slot] = order
    col_local = np.full(NT * S, 128.0, np.float32)
    col_local[slot] = (col_s & 127).astype(np.float32)
    payload = ea[slot_edge]  # [NT*S, H]

    x_pad = np.zeros((NPAD, H), np.float32)
    x_pad[:N] = np.asarray(x, dtype=np.float32)

    per_core = []
    for c in range(NC):
        r0, r1 = c * TPC * S, (c + 1) * TPC * S
        pay_c = np.ascontiguousarray(
            payload[r0:r1]
            .reshape(TPC, K, P, H)
            .transpose(0, 2, 1, 3)
            .reshape(TPC * P, K * H)
            .astype(ml_dtypes.bfloat16)
        )
        col_c = np.ascontiguousarray(
            col_local[r0:r1].reshape(TPC, K, P).transpose(2, 0, 1).reshape(P, TPC * K)
        )
        xt_c = np.ascontiguousarray(
            x_pad[c * NPC : (c + 1) * NPC]
            .reshape(TPC, P, H)
            .transpose(0, 2, 1)
            .reshape(TPC * P, P)
            .astype(ml_dtypes.bfloat16)
        )
        per_core.append((pay_c, col_c, xt_c))
    return K, per_core


def _build_program(K):
    # Bacc (not raw Bass): its compile pass splits multi-semaphore waits into
    # event-semaphore chains — walrus codegen allows only 1 wait per
    # instruction on this toolchain.
    nc = _Bacc("TRN2", target_bir_lowering=False, debug=False, num_devices=NC)

    edges_h = nc.dram_tensor("edges", [TPC * P, K * P], BF16, kind="ExternalInput")
    cols_h = nc.dram_tensor("cols", [P, TPC * K], F32, kind="ExternalInput")
    xt_h = nc.dram_tensor("xt", [TPC * P, P], BF16, kind="ExternalInput")
    w_h = {
        name: nc.dram_tensor(name, [P, P], BF16, kind="ExternalInput")
        for name in ("w1a", "w1b", "w2", "w3")
    }
    # b1,b2,b3,g1,g2,g3,be1,be2,be3 packed as columns of one tensor (one DMA,
    # one semaphore for every per-partition vector operand).
    vecs_h = nc.dram_tensor("vecs", [P, 9], F32, kind="ExternalInput")
    iota_h = nc.dram_tensor("iota", [P, P], F32, kind="ExternalInput")
    out_h = nc.dram_tensor("out", [TPC * P, P], F32, kind="ExternalOutput")
    VIDX = {n: i for i, n in enumerate(("b1", "b2", "b3", "g1", "g2", "g3", "be1", "be2", "be3"))}

    with tile_mod.TileContext(nc) as tc:
        with (
            tc.tile_pool(name="const", bufs=1) as cpool,
            tc.tile_pool(name="edges", bufs=3) as epool,
            tc.tile_pool(name="xin", bufs=3) as xpool,
            tc.tile_pool(name="sel", bufs=4) as selpool,
            tc.tile_pool(name="work", bufs=3) as wpool,
            tc.tile_pool(name="stats", bufs=6) as spool,
            tc.tile_pool(name="psum", bufs=8, space="PSUM") as ppool,
        ):
            ident = cpool.tile([P, P], F32)
            make_identity(nc, ident[:])

            def transpose(dst_psum, src_sbuf):
                nc.tensor.transpose(dst_psum[:], src_sbuf[:], ident[:])
            iota = cpool.tile_from(iota_h[:])
            cols = cpool.tile_from(cols_h[:])
            W = {k: cpool.tile_from(h[:], name=f"w_{k}") for k, h in w_h.items()}
            vecs = cpool.tile_from(vecs_h[:])
            V = {n: vecs[:, i : i + 1] for n, i in VIDX.items()}
            eps = cpool.tile([P, 1], F32)
            nc.gpsimd.memset(eps[:], 1e-5)
            half = cpool.tile([P, 1], F32)
            nc.gpsimd.memset(half[:], 0.5)

            def layer(zT_psum, b, g, be, out_dtype=BF16):
                """zT_psum: [h_out, n] pre-activation in PSUM.
                Returns ssp(LN(zT + b) * g + be) as [h_out, n] in SBUF,
                including the -log2 shift (ln(0.5*exp(y) + 0.5))."""
                # NOTE: TensorScalar's ISA struct fits only ONE sync wait, so
                # everything here uses tensor_tensor with broadcast [P,1] APs.
                zbT = wpool.tile([P, P], F32, tag="zbT")
                nc.vector.tensor_tensor(
                    zbT[:], zT_psum[:], V[b].to_broadcast([P, P]),
                    op=mybir.AluOpType.add,
                )
                z_rm = ppool.tile([P, P], F32, tag="ps")
                transpose(z_rm, zbT)
                st6 = spool.tile([P, 6], F32, tag="st6")
                nc.vector.bn_stats(st6[:], z_rm[:])
                st2 = spool.tile([P, 2], F32, tag="st2")
                nc.vector.bn_aggr(st2[:], st6[:])
                # rsqrt(var + eps) = exp(-0.5 * ln(var + eps)); no ACT func
                # set holds both Sqrt and a softplus path, but Ln+Exp coexist.
                lnv = spool.tile([P, 1], F32, tag="lnv")
                nc.scalar.activation(
                    lnv[:], st2[:, 1:2], mybir.ActivationFunctionType.Ln,
                    bias=eps[:, 0:1],
                )
                rsig = spool.tile([P, 1], F32, tag="rsig")
                nc.scalar.activation(
                    rsig[:], lnv[:], mybir.ActivationFunctionType.Exp, scale=-0.5
                )
                zc = wpool.tile([P, P], F32, tag="zc")
                nc.vector.tensor_tensor(
                    zc[:], z_rm[:], st2[:, 0:1].to_broadcast([P, P]),
                    op=mybir.AluOpType.subtract,
                )
                zn = wpool.tile([P, P], F32, tag="zn")
                zn_eng = nc.gpsimd if os.environ.get("KERNEL_ZN_GPS", "1") == "1" else nc.vector
                zn_eng.tensor_tensor(
                    zn[:], zc[:], rsig[:, 0:1].to_broadcast([P, P]),
                    op=mybir.AluOpType.mult,
                )
                znT = ppool.tile([P, P], F32, tag="ps")
                transpose(znT, zn)
                # ssp(y) = softplus(y) - log2 = ln(0.5*exp(y) + 0.5), with
                # y = g*zn + be. LN output is bounded (|zn| <= sqrt(127)) so
                # exp cannot overflow.
                ez = wpool.tile([P, P], F32, tag="ez")
                nc.scalar.activation(
                    ez[:],
                    znT[:],
                    mybir.ActivationFunctionType.Exp,
                    bias=V[be],
                    scale=V[g],
                )
                spT = wpool.tile([P, P], out_dtype, tag="spT")
                nc.scalar.activation(
                    spT[:], ez[:], mybir.ActivationFunctionType.Ln,
                    bias=half[:, 0:1], scale=0.5,
                )
                return spT

            sel_eng = nc.gpsimd if os.environ.get("KERNEL_SEL_GPS", "0") == "1" else nc.vector
            n_tiles = int(os.environ.get("KERNEL_TPC", str(TPC)))
            for t in range(n_tiles):
                ed = epool.tile([P, K * P], BF16, tag="ed")
                nc.sync.dma_start(out=ed[:], in_=edges_h[t * P : (t + 1) * P, :])
                xt = xpool.tile([P, P], BF16, tag="xt")
                nc.sync.dma_start(out=xt[:], in_=xt_h[t * P : (t + 1) * P, :])

                aggT = ppool.tile([P, P], F32, tag="ps")
                for k in range(K):
                    sel = selpool.tile([P, P], BF16, tag="sel")
                    sel_eng.tensor_tensor(
                        sel[:],
                        cols[:, t * K + k : t * K + k + 1].to_broadcast([P, P]),
                        iota[:],
                        op=mybir.AluOpType.is_equal,
                    )
                    nc.tensor.matmul(
                        out=aggT[:],
                        lhsT=ed[:, k * P : (k + 1) * P],
                        rhs=sel[:],
                        start=(k == 0),
                        stop=(k == K - 1),
                    )
                aggS = wpool.tile([P, P], BF16, tag="aggS")
                nc.vector.tensor_copy(aggS[:], aggT[:])

                z1T = ppool.tile([P, P], F32, tag="ps")
                nc.tensor.matmul(out=z1T[:], lhsT=W["w1a"][:], rhs=xt[:], start=True, stop=False)
                nc.tensor.matmul(out=z1T[:], lhsT=W["w1b"][:], rhs=aggS[:], start=False, stop=True)
                h1T = layer(z1T, "b1", "g1", "be1")

                z2T = ppool.tile([P, P], F32, tag="ps")
                nc.tensor.matmul(out=z2T[:], lhsT=W["w2"][:], rhs=h1T[:], start=True, stop=True)
                h2T = layer(z2T, "b2", "g2", "be2")

                z3T = ppool.tile([P, P], F32, tag="ps")
                nc.tensor.matmul(out=z3T[:], lhsT=W["w3"][:], rhs=h2T[:], start=True, stop=True)
                h3T = layer(z3T, "b3", "g3", "be3", out_dtype=F32)
                nc.sync.dma_start(out=out_h[t * P : (t + 1) * P, :], in_=h3T[:])

    if not nc.is_finalized():
        nc.finalize()
    return nc


def kernel(
    x, edge_index, edge_attr,
    W1, b1, g1, be1, W2, b2, g2, be2, W3, b3, g3, be3,
):
    global LAST_RESULT
    W1 = np.asarray(W1, np.float32)
    W2 = np.asarray(W2, np.float32)
    W3 = np.asarray(W3, np.float32)

    K, per_core = _host_prep(x, edge_index, edge_attr)
    nc = _build_program(K)

    vecs = np.stack(
        [np.asarray(v, np.float32) for v in (b1, b2, b3, g1, g2, g3, be1, be2, be3)],
        axis=1,
    )  # [128, 9], column order must match VIDX in _build_program
    shared = {
        "w1a": np.ascontiguousarray(W1[:P]).astype(ml_dtypes.bfloat16),
        "w1b": np.ascontiguousarray(W1[P:]).astype(ml_dtypes.bfloat16),
        "w2": W2.astype(ml_dtypes.bfloat16),
        "w3": W3.astype(ml_dtypes.bfloat16),
        "vecs": np.ascontiguousarray(vecs),
        "iota": np.ascontiguousarray(
            np.broadcast_to(np.arange(P, dtype=np.float32), (P, P))
        ),
    }
    in_maps = [
        {"edges": pay_c, "cols": col_c, "xt": xt_c, **shared}
        for (pay_c, col_c, xt_c) in per_core
    ]

    trace = bool(int(os.environ.get("KERNEL_TRACE", "0")))
    res = run_bass_kernel_spmd(nc, in_maps, core_ids=list(range(NC)), trace=trace)
    LAST_RESULT = res

    out = np.concatenate(
        [
            r["out"].reshape(TPC, P, P).transpose(0, 2, 1).reshape(NPC, H)
            for r in res.results
        ],
        axis=0,
    )
    return np.ascontiguousarray(out[:N])



# revision 2
# speedup vs baseline: 2.8948x; 2.8948x over previous
"""Trainium2 Bass kernel for nn_NodeModel (GNN message passing + 3-layer node MLP).

Strategy (node-parallel, 8 cores, 512-node groups):
  - Host: sort edges by destination node, bucket into 128-node tiles, pad each
    tile's edge list to K chunks of 128 edges. Nodes sharded contiguously
    across 8 cores (12800 padded nodes each = 25 groups of 512).
  - Device (per core, per 512-node group), all activations feature-major
    [h, node] so no PE transposes are needed:
      * scatter: sel chunk one-hot built via DVE tensor_scalar(is_equal)
        (iota vs per-partition col scalar, 4x DVE mode); aggT accumulated in
        a [128, 512] PSUM bank via K matmuls per 128-node tile.
      * MLP with HOST-CENTERED weights: W~ = W - mean_out(W), b~ = b - mean(b)
        makes z~ = W~ h + b~ zero-mean over features, so LayerNorm needs no
        mean subtraction on device.
      * var per node = ones^T (z~*z~) via one matmul -> s [1, 512] PSUM;
        rsig = Exp(-0.5*Ln(s/H + eps)) on ACT ([1,512] rows);
        broadcast to [128,512] via gpsimd partition_broadcast;
        zn = z~ * R on DVE; ssp fused as Ln(0.5*Exp(g*zn+be)+0.5) on ACT.
"""

import os
import sys

import numpy as np

sys.path.insert(0, "/opt/trn_rl_repo")

import bass_rust as _bass_rust
import ml_dtypes

from concourse import bacc, bass, hw_specs, mybir
from concourse import tile as tile_mod
from concourse.bass_utils import run_bass_kernel_spmd


class _Bacc(bacc.Bacc):
    """Bacc with the ACT table chooser pinned to the single function set
    that holds Ln+Exp (the only funcs we use). The default greedy chooser
    can alternate sets, costing ~1.3us ACT_TABLE_LOAD per switch."""

    def insert_act_table_loads(self):
        has_activation = any(
            isinstance(i, mybir.InstActivation)
            for b in self.main_func.blocks
            for i in b.instructions
        )
        if not has_activation:
            return
        keep = "natural_log_exp_and_others"
        tables = [
            (n, (s if n == keep else set()))
            for n, s in hw_specs.get_activation_tables(self.m.arch).items()
        ]
        _bass_rust.insert_act_table_loads(self, tables)


N, E, H = 100000, 600000, 128
NC = 8
P = 128
GRP = 4                  # 128-node tiles per group
F = GRP * P              # group free width (512)
TPC = 100                # 128-node tiles per core
G = TPC // GRP           # groups per core (25)
NPC = TPC * P            # nodes per core (12800)
NPAD = NPC * NC          # padded node count (102400)
NT = NPAD // P           # total node tiles (800)

F32 = mybir.dt.float32
BF16 = mybir.dt.bfloat16

LAST_RESULT = None  # BassKernelResults of the most recent run (for profiling)


def _host_prep(x, edge_index, edge_attr):
    col = np.asarray(edge_index)[1].astype(np.int64)
    ea = np.ascontiguousarray(np.asarray(edge_attr, dtype=np.float32))
    order = np.argsort(col, kind="stable")
    col_s = col[order]
    tile_of = col_s >> 7
    counts = np.bincount(tile_of, minlength=NT)
    K = int(np.ceil(counts.max() / P))
    S = K * P
    starts = np.zeros(NT + 1, np.int64)
    starts[1:] = np.cumsum(counts)
    pos = np.arange(E) - starts[tile_of]
    slot = tile_of * S + pos
    slot_edge = np.zeros(NT * S, np.int64)
    slot_edge[slot] = order
    col_local = np.full(NT * S, 128.0, np.float32)
    col_local[slot] = (col_s & 127).astype(np.float32)
    payload = ea[slot_edge]  # [NT*S, H]; padded slots point at edge 0 but
    # their col_local=128 never matches iota, so they contribute nothing.

    x_pad = np.zeros((NPAD, H), np.float32)
    x_pad[:N] = np.asarray(x, dtype=np.float32)

    per_core = []
    for c in range(NC):
        r0, r1 = c * TPC * S, (c + 1) * TPC * S
        # edges: [G*P(e), GRP*K*P(b,k,h)] bf16, group-major rows
        pay_c = np.ascontiguousarray(
            payload[r0:r1]
            .reshape(G, GRP, K, P, H)
            .transpose(0, 3, 1, 2, 4)
            .reshape(G * P, GRP * K * P)
            .astype(ml_dtypes.bfloat16)
        )
        # cols: [P(e), TPC*K] f32, tile-major
        col_c = np.ascontiguousarray(
            col_local[r0:r1].reshape(TPC, K, P).transpose(2, 0, 1).reshape(P, TPC * K)
        )
        # xt: [G*P(h), GRP*P(n)] bf16 — x transposed per group
        xt_c = np.ascontiguousarray(
            x_pad[c * NPC : (c + 1) * NPC]
            .reshape(G, GRP, P, H)
            .transpose(0, 3, 1, 2)
            .reshape(G * P, F)
            .astype(ml_dtypes.bfloat16)
        )
        per_core.append((pay_c, col_c, xt_c))
    return K, per_core


def _build_program(K):
    nc = _Bacc("TRN2", target_bir_lowering=False, debug=False, num_devices=NC)

    edges_h = nc.dram_tensor("edges", [G * P, GRP * K * P], BF16, kind="ExternalInput")
    cols_h = nc.dram_tensor("cols", [P, TPC * K], F32, kind="ExternalInput")
    xt_h = nc.dram_tensor("xt", [G * P, F], BF16, kind="ExternalInput")
    w_h = {
        name: nc.dram_tensor(name, [P, P], BF16, kind="ExternalInput")
        for name in ("w1a", "w1b", "w2", "w3")
    }
    # b~1..3 (centered), g1..3, be1..3, eps, half packed as columns.
    vecs_h = nc.dram_tensor("vecs", [P, 11], F32, kind="ExternalInput")
    iota_h = nc.dram_tensor("iota", [P, P], BF16, kind="ExternalInput")
    ones_h = nc.dram_tensor("ones", [P, 1], BF16, kind="ExternalInput")
    out_h = nc.dram_tensor("out", [G * P, F], F32, kind="ExternalOutput")
    VIDX = {
        n: i
        for i, n in enumerate(
            ("b1", "b2", "b3", "g1", "g2", "g3", "be1", "be2", "be3", "eps", "half")
        )
    }

    with tile_mod.TileContext(nc) as tc:
        with (
            tc.tile_pool(name="const", bufs=1) as cpool,
            tc.tile_pool(name="edges", bufs=3) as epool,
            tc.tile_pool(name="xin", bufs=3) as xpool,
            tc.tile_pool(name="sel", bufs=8) as selpool,
            tc.tile_pool(name="work", bufs=3) as wpool,
            tc.tile_pool(name="rbuf", bufs=3) as rpool,
            tc.tile_pool(name="stats", bufs=4) as spool,
            tc.tile_pool(name="ps_agg", bufs=2, space="PSUM") as apool,
            tc.tile_pool(name="ps_z", bufs=3, space="PSUM") as zpool,
            tc.tile_pool(name="ps_s", bufs=3, space="PSUM") as stpool,
        ):
            iota = cpool.tile_from(iota_h[:])
            cols = cpool.tile_from(cols_h[:])
            ones = cpool.tile_from(ones_h[:])
            W = {k: cpool.tile_from(h[:], name=f"w_{k}") for k, h in w_h.items()}
            vecs = cpool.tile_from(vecs_h[:])
            V = {n: vecs[:, i : i + 1] for n, i in VIDX.items()}

            def layer(z_psum, b, g, be, out_dtype=BF16):
                """z_psum: [h_out, n] pre-activation (centered, no bias) in
                PSUM. Returns ssp(LN(z+b~)*g+be) as [h_out, n] in SBUF."""
                zt = wpool.tile([P, F], BF16, tag="zt")
                nc.vector.tensor_scalar(
                    zt[:], z_psum[:], V[b], None, op0=mybir.AluOpType.add
                )
                q = wpool.tile([P, F], BF16, tag="q")
                nc.vector.tensor_tensor(q[:], zt[:], zt[:], op=mybir.AluOpType.mult)
                s = stpool.tile([1, F], F32, tag="s")
                nc.tensor.matmul(out=s[:], lhsT=ones[:], rhs=q[:], start=True, stop=True)
                u = spool.tile([1, F], F32, tag="u")
                nc.scalar.activation(
                    u[:], s[:], mybir.ActivationFunctionType.Ln,
                    bias=V["eps"][0:1, :], scale=1.0 / H,
                )
                rsig = spool.tile([1, F], BF16, tag="rs")
                nc.scalar.activation(
                    rsig[:], u[:], mybir.ActivationFunctionType.Exp, scale=-0.5
                )
                R = rpool.tile([P, F], BF16, tag="R")
                nc.gpsimd.partition_broadcast(R[:], rsig[:])
                zn = wpool.tile([P, F], BF16, tag="zn")
                nc.vector.tensor_tensor(zn[:], zt[:], R[:], op=mybir.AluOpType.mult)
                ez = wpool.tile([P, F], F32, tag="ez")
                nc.scalar.activation(
                    ez[:], zn[:], mybir.ActivationFunctionType.Exp,
                    bias=V[be], scale=V[g],
                )
                sp = wpool.tile([P, F], out_dtype, tag="sp")
                nc.scalar.activation(
                    sp[:], ez[:], mybir.ActivationFunctionType.Ln,
                    bias=V["half"], scale=0.5,
                )
                return sp

            for gi in range(G):
                ed = epool.tile([P, GRP * K * P], BF16, tag="ed")
                nc.sync.dma_start(out=ed[:], in_=edges_h[gi * P : (gi + 1) * P, :])
                xt = xpool.tile([P, F], BF16, tag="xt")
                nc.sync.dma_start(out=xt[:], in_=xt_h[gi * P : (gi + 1) * P, :])

                agg = apool.tile([P, F], F32, tag="agg")
                for b in range(GRP):
                    t = gi * GRP + b
                    sel = selpool.tile([P, K * P], BF16, tag="sel")
                    for k in range(K):
                        nc.vector.tensor_scalar(
                            sel[:, k * P : (k + 1) * P],
                            iota[:],
                            cols[:, t * K + k : t * K + k + 1],
                            None,
                            op0=mybir.AluOpType.is_equal,
                        )
                    for k in range(K):
                        nc.tensor.matmul(
                            out=agg[:, b * P : (b + 1) * P],
                            lhsT=ed[:, (b * K + k) * P : (b * K + k + 1) * P],
                            rhs=sel[:, k * P : (k + 1) * P],
                            start=(k == 0),
                            stop=(k == K - 1),
                        )
                aggS = wpool.tile([P, F], BF16, tag="aggS")
                nc.vector.tensor_copy(aggS[:], agg[:])

                z1 = zpool.tile([P, F], F32, tag="z")
                nc.tensor.matmul(out=z1[:], lhsT=W["w1a"][:], rhs=xt[:], start=True, stop=False)
                nc.tensor.matmul(out=z1[:], lhsT=W["w1b"][:], rhs=aggS[:], start=False, stop=True)
                h1 = layer(z1, "b1", "g1", "be1")

                z2 = zpool.tile([P, F], F32, tag="z")
                nc.tensor.matmul(out=z2[:], lhsT=W["w2"][:], rhs=h1[:], start=True, stop=True)
                h2 = layer(z2, "b2", "g2", "be2")

                z3 = zpool.tile([P, F], F32, tag="z")
                nc.tensor.matmul(out=z3[:], lhsT=W["w3"][:], rhs=h2[:], start=True, stop=True)
                h3 = layer(z3, "b3", "g3", "be3", out_dtype=F32)
                nc.sync.dma_start(out=out_h[gi * P : (gi + 1) * P, :], in_=h3[:])

    if not nc.is_finalized():
        nc.finalize()
    return nc


def kernel(
    x, edge_index, edge_attr,
    W1, b1, g1, be1, W2, b2, g2, be2, W3, b3, g3, be3,
):
    global LAST_RESULT
    W1 = np.asarray(W1, np.float32)
    W2 = np.asarray(W2, np.float32)
    W3 = np.asarray(W3, np.float32)
    # Center weights/biases over the OUTPUT feature axis so z~ = W~h + b~ is
    # exactly zero-mean across features => LayerNorm mean subtraction is free.
    W1c = W1 - W1.mean(axis=1, keepdims=True)
    W2c = W2 - W2.mean(axis=1, keepdims=True)
    W3c = W3 - W3.mean(axis=1, keepdims=True)
    b1c = np.asarray(b1, np.float32) - np.float32(np.mean(b1))
    b2c = np.asarray(b2, np.float32) - np.float32(np.mean(b2))
    b3c = np.asarray(b3, np.float32) - np.float32(np.mean(b3))

    K, per_core = _host_prep(x, edge_index, edge_attr)
    nc = _build_program(K)

    eps_col = np.full((P,), 1e-5, np.float32)
    half_col = np.full((P,), 0.5, np.float32)
    vecs = np.stack(
        [b1c, b2c, b3c]
        + [np.asarray(v, np.float32) for v in (g1, g2, g3, be1, be2, be3)]
        + [eps_col, half_col],
        axis=1,
    )  # [128, 11], column order must match VIDX in _build_program
    shared = {
        "w1a": np.ascontiguousarray(W1c[:P]).astype(ml_dtypes.bfloat16),
        "w1b": np.ascontiguousarray(W1c[P:]).astype(ml_dtypes.bfloat16),
        "w2": W2c.astype(ml_dtypes.bfloat16),
        "w3": W3c.astype(ml_dtypes.bfloat16),
        "vecs": np.ascontiguousarray(vecs),
        "iota": np.ascontiguousarray(
            np.broadcast_to(np.arange(P, dtype=np.float32), (P, P))
        ).astype(ml_dtypes.bfloat16),
        "ones": np.ones((P, 1), ml_dtypes.bfloat16),
    }
    in_maps = [
        {"edges": pay_c, "cols": col_c, "xt": xt_c, **shared}
        for (pay_c, col_c, xt_c) in per_core
    ]

    trace = bool(int(os.environ.get("KERNEL_TRACE", "0")))
    res = run_bass_kernel_spmd(nc, in_maps, core_ids=list(range(NC)), trace=trace)
    LAST_RESULT = res

    out = np.concatenate(
        [
            r["out"].reshape(G, P, GRP, P).transpose(0, 2, 3, 1).reshape(NPC, H)
            for r in res.results
        ],
        axis=0,
    )
    return np.ascontiguousarray(out[:N])


# revision 8
# speedup vs baseline: 3.1019x; 1.0715x over previous
"""Trainium2 Bass kernel for nn_NodeModel (GNN message passing + 3-layer node MLP).

Strategy (node-parallel, 8 cores, 512-node groups, group PAIRS in lockstep):
  - Host: sort edges by destination node, bucket into 128-node tiles, pad each
    tile's edge list to K chunks of 128 edges. Nodes sharded contiguously
    across 8 cores (12800 padded nodes each = 25 groups of 512).
  - Device (per core), all activations feature-major [h, node] (no PE
    transposes):
      * scatter: one-hot sel per tile via a single DVE is_equal with a
        3D broadcast AP (cols [P,K,1] vs iota [P,K,P]); aggT accumulated in a
        [128, 512] PSUM bank via K matmuls per 128-node tile.
      * HOST-CENTERED weights: W~ = W - mean_out(W), b~ = b - mean(b) make
        z~ = W~h + b~ zero-mean over features => LN mean subtraction is free.
      * variance: per group-pair, two ones-matmuls accumulate into ONE
        [2, 512] PSUM tile (lhsT = indicator column trick), so the [1,n]-
        shaped Ln/Exp ACT ops are shared by both groups of the pair.
      * rsig broadcast [1,512]->[128,512] via DRAM round-trip DMA (SBUF
        partition-broadcast DMA is not expressible; gpsimd partition_broadcast
        saturates the Q7 queue with multi-us semaphore ops).
      * layers 1,3: q = ACT Square(z~raw + b~); zn = DVE scalar_tensor_tensor
        (z~raw + b~) * R. layer 2: z~ materialized bf16 on DVE; q and zn are
        2x-mode DVE tensor_tensor. Balances ACT vs DVE queue time.
      * ssp fused as Ln(0.5*Exp(g*zn+be)+0.5) on ACT (exact: softplus - log2).
"""

import os
import sys

import numpy as np

sys.path.insert(0, "/opt/trn_rl_repo")

import bass_rust as _bass_rust
import ml_dtypes

from concourse import bacc, bass, hw_specs, mybir
from concourse import tile as tile_mod
from concourse.bass_utils import run_bass_kernel_spmd


class _Bacc(bacc.Bacc):
    """Bacc with the ACT table chooser pinned to the single function set
    that holds Ln+Exp+Square (the only funcs we use)."""

    def insert_act_table_loads(self):
        has_activation = any(
            isinstance(i, mybir.InstActivation)
            for b in self.main_func.blocks
            for i in b.instructions
        )
        if not has_activation:
            return
        keep = "natural_log_exp_and_others"
        tables = [
            (n, (s if n == keep else set()))
            for n, s in hw_specs.get_activation_tables(self.m.arch).items()
        ]
        _bass_rust.insert_act_table_loads(self, tables)


N, E, H = 100000, 600000, 128
NC = 8
P = 128
GRP = 4                  # 128-node tiles per group
F = GRP * P              # group free width (512)
TPC = 100                # 128-node tiles per core
G = TPC // GRP           # groups per core (25)
NPC = TPC * P            # nodes per core (12800)
NPAD = NPC * NC          # padded node count (102400)
NT = NPAD // P           # total node tiles (800)

F32 = mybir.dt.float32
BF16 = mybir.dt.bfloat16
ALU = mybir.AluOpType
AF = mybir.ActivationFunctionType

LAST_RESULT = None  # BassKernelResults of the most recent run (for profiling)


def _host_prep(x, edge_index, edge_attr):
    col = np.asarray(edge_index)[1].astype(np.int64)
    ea = np.ascontiguousarray(np.asarray(edge_attr, dtype=np.float32))
    order = np.argsort(col, kind="stable")
    col_s = col[order]
    tile_of = col_s >> 7
    counts = np.bincount(tile_of, minlength=NT)
    K = int(np.ceil(counts.max() / P))
    S = K * P
    starts = np.zeros(NT + 1, np.int64)
    starts[1:] = np.cumsum(counts)
    pos = np.arange(E) - starts[tile_of]
    slot = tile_of * S + pos
    slot_edge = np.zeros(NT * S, np.int64)
    slot_edge[slot] = order
    col_local = np.full(NT * S, 128.0, np.float32)
    col_local[slot] = (col_s & 127).astype(np.float32)
    payload = ea[slot_edge]  # padded slots alias edge 0 but col=128 never
    # matches iota, so they contribute nothing.

    x_pad = np.zeros((NPAD, H), np.float32)
    x_pad[:N] = np.asarray(x, dtype=np.float32)

    per_core = []
    for c in range(NC):
        r0, r1 = c * TPC * S, (c + 1) * TPC * S
        pay_c = np.ascontiguousarray(
            payload[r0:r1]
            .reshape(G, GRP, K, P, H)
            .transpose(0, 3, 1, 2, 4)
            .reshape(G * P, GRP * K * P)
            .astype(ml_dtypes.bfloat16)
        )
        col_c = np.ascontiguousarray(
            col_local[r0:r1].reshape(TPC, K, P).transpose(2, 0, 1).reshape(P, TPC * K)
        ).astype(ml_dtypes.bfloat16)
        xt_c = np.ascontiguousarray(
            x_pad[c * NPC : (c + 1) * NPC]
            .reshape(G, GRP, P, H)
            .transpose(0, 3, 1, 2)
            .reshape(G * P, F)
            .astype(ml_dtypes.bfloat16)
        )
        per_core.append((pay_c, col_c, xt_c))
    return K, per_core


def _build_program(K):
    nc = _Bacc("TRN2", target_bir_lowering=False, debug=False, num_devices=NC)

    edges_h = nc.dram_tensor("edges", [G * P, GRP * K * P], BF16, kind="ExternalInput")
    cols_h = nc.dram_tensor("cols", [P, TPC * K], BF16, kind="ExternalInput")
    xt_h = nc.dram_tensor("xt", [G * P, F], BF16, kind="ExternalInput")
    w_h = {
        name: nc.dram_tensor(name, [P, P], BF16, kind="ExternalInput")
        for name in ("w1a", "w1b", "w2", "w3")
    }
    # b~1..3 (centered), g1..3, be1..3, eps, half as columns.
    vecs_h = nc.dram_tensor("vecs", [P, 11], F32, kind="ExternalInput")
    iota_h = nc.dram_tensor("iota", [P, K * P], BF16, kind="ExternalInput")
    # epick: [1,0,1] columns -> lhsT slices [:,0:2]=e0, [:,1:3]=e1 for the
    # paired variance reduce; [:,0:1] = plain ones column.
    ep_h = nc.dram_tensor("epick", [P, 3], BF16, kind="ExternalInput")
    out_h = nc.dram_tensor("out", [G * P, F], F32, kind="ExternalOutput")
    # rsig spill rows, one pair of rows per (pair, layer)
    NPAIR = (G + 1) // 2
    scr_h = nc.dram_tensor("scr", [NPAIR * 3 * 2, F], BF16, kind="Internal")
    VIDX = {
        n: i
        for i, n in enumerate(
            ("b1", "b2", "b3", "g1", "g2", "g3", "be1", "be2", "be3", "eps", "half")
        )
    }

    with tile_mod.TileContext(nc) as tc:
        with (
            tc.tile_pool(name="const", bufs=1) as cpool,
            tc.tile_pool(name="edges", bufs=3) as epool,
            tc.tile_pool(name="xin", bufs=4) as xpool,
            tc.tile_pool(name="sel", bufs=8) as selpool,
            tc.tile_pool(name="work", bufs=4) as wpool,
            tc.tile_pool(name="rbuf", bufs=6) as rpool,
            tc.tile_pool(name="stats", bufs=4) as spool,
            tc.tile_pool(name="ps_agg", bufs=2, space="PSUM") as apool,
            tc.tile_pool(name="ps_z", bufs=4, space="PSUM") as zpool,
            tc.tile_pool(name="ps_s", bufs=2, space="PSUM") as stpool,
        ):
            iota = cpool.tile_from(iota_h[:])
            cols = cpool.tile_from(cols_h[:])
            epick = cpool.tile_from(ep_h[:])
            W = {k: cpool.tile_from(h[:], name=f"w_{k}") for k, h in w_h.items()}
            vecs = cpool.tile_from(vecs_h[:])
            V = {n: vecs[:, i : i + 1] for n, i in VIDX.items()}

            iota3 = iota[:].rearrange("p (k n) -> p k n", k=K)

            def scatter(gi):
                """DMA edges/x and build agg [h, F] in PSUM for group gi.
                Returns (aggS bf16 SBUF, xt bf16 SBUF)."""
                ed = epool.tile([P, GRP * K * P], BF16, tag="ed")
                nc.sync.dma_start(out=ed[:], in_=edges_h[gi * P : (gi + 1) * P, :])
                xt = xpool.tile([P, F], BF16, tag="xt")
                nc.sync.dma_start(out=xt[:], in_=xt_h[gi * P : (gi + 1) * P, :])
                agg = apool.tile([P, F], F32, tag="agg")
                for b in range(GRP):
                    t = gi * GRP + b
                    sel = selpool.tile([P, K * P], BF16, tag="sel")
                    nc.vector.tensor_tensor(
                        sel[:].rearrange("p (k n) -> p k n", k=K),
                        cols[:, t * K : (t + 1) * K].unsqueeze(2).broadcast_to([P, K, P]),
                        iota3,
                        op=ALU.is_equal,
                    )
                    for k in range(K):
                        nc.tensor.matmul(
                            out=agg[:, b * P : (b + 1) * P],
                            lhsT=ed[:, (b * K + k) * P : (b * K + k + 1) * P],
                            rhs=sel[:, k * P : (k + 1) * P],
                            start=(k == 0),
                            stop=(k == K - 1),
                        )
                aggS = wpool.tile([P, F], BF16, tag="aggS")
                nc.vector.tensor_copy(aggS[:], agg[:])
                return aggS, xt

            def layer_pair(zs, li, pidx, b, g, be, out_dtype=BF16):
                """zs: list of (PSUM z~raw [h,F]) for the groups of this pair.
                li: layer index (0-based), pidx: pair index (scratch rows).
                Returns list of ssp outputs."""
                npair = len(zs)
                mid = li == 1  # layer 2: DVE-heavy variant (z~ materialized)
                zts, qs = [], []
                for z in zs:
                    if mid:
                        zt = wpool.tile([P, F], BF16, tag="zt")
                        nc.vector.tensor_scalar(zt[:], z[:], V[b], None, op0=ALU.add)
                        q = wpool.tile([P, F], BF16, tag="q")
                        nc.vector.tensor_tensor(q[:], zt[:], zt[:], op=ALU.mult)
                        zts.append(zt)
                    else:
                        q = wpool.tile([P, F], BF16, tag="q")
                        nc.scalar.activation(q[:], z[:], AF.Square, bias=V[b])
                        zts.append(None)
                    qs.append(q)
                nr = 2 if npair == 2 else 1
                s2 = stpool.tile([2, F], F32, tag="s2")
                for j, q in enumerate(qs):
                    nc.tensor.matmul(
                        out=s2[:nr, :],
                        lhsT=epick[:, j : j + 2] if npair == 2 else epick[:, 0:1],
                        rhs=q[:],
                        start=(j == 0),
                        stop=(j == npair - 1),
                    )
                u2 = spool.tile([2, F], F32, tag="u2")
                nc.scalar.activation(
                    u2[:nr, :], s2[:nr, :], AF.Ln, bias=V["eps"][0:nr, :], scale=1.0 / H
                )
                rs2 = spool.tile([2, F], BF16, tag="rs2")
                nc.scalar.activation(rs2[:nr, :], u2[:nr, :], AF.Exp, scale=-0.5)
                row = (pidx * 3 + li) * 2  # unique scratch rows
                nc.sync.dma_start(out=scr_h[row : row + nr, :], in_=rs2[:nr, :])
                outs = []
                for j, z in enumerate(zs):
                    R = rpool.tile([P, F], BF16, tag="R")
                    nc.sync.dma_start(
                        out=R[:], in_=scr_h[row + j : row + j + 1, :].to_broadcast([P, F])
                    )
                    zn = wpool.tile([P, F], BF16, tag="zn")
                    if mid:
                        nc.vector.tensor_tensor(zn[:], zts[j][:], R[:], op=ALU.mult)
                    else:
                        nc.vector.scalar_tensor_tensor(
                            zn[:], z[:], V[b], R[:], op0=ALU.add, op1=ALU.mult
                        )
                    ez = wpool.tile([P, F], F32, tag="ez")
                    nc.scalar.activation(ez[:], zn[:], AF.Exp, bias=V[be], scale=V[g])
                    sp = wpool.tile([P, F], out_dtype, tag="sp")
                    nc.scalar.activation(sp[:], ez[:], AF.Ln, bias=V["half"], scale=0.5)
                    outs.append(sp)
                return outs

            for p0 in range(0, G, 2):
                pair = [p0] if p0 + 1 >= G else [p0, p0 + 1]
                pidx = p0 // 2
                ins = [scatter(gi) for gi in pair]
                zs = []
                for (aggS, xt) in ins:
                    z1 = zpool.tile([P, F], F32, tag="z")
                    nc.tensor.matmul(out=z1[:], lhsT=W["w1a"][:], rhs=xt[:], start=True, stop=False)
                    nc.tensor.matmul(out=z1[:], lhsT=W["w1b"][:], rhs=aggS[:], start=False, stop=True)
                    zs.append(z1)
                h1s = layer_pair(zs, 0, pidx, "b1", "g1", "be1")
                zs2 = []
                for h1 in h1s:
                    z2 = zpool.tile([P, F], F32, tag="z")
                    nc.tensor.matmul(out=z2[:], lhsT=W["w2"][:], rhs=h1[:], start=True, stop=True)
                    zs2.append(z2)
                h2s = layer_pair(zs2, 1, pidx, "b2", "g2", "be2")
                zs3 = []
                for h2 in h2s:
                    z3 = zpool.tile([P, F], F32, tag="z")
                    nc.tensor.matmul(out=z3[:], lhsT=W["w3"][:], rhs=h2[:], start=True, stop=True)
                    zs3.append(z3)
                h3s = layer_pair(zs3, 2, pidx, "b3", "g3", "be3", out_dtype=F32)
                for j, gi in enumerate(pair):
                    nc.sync.dma_start(
                        out=out_h[gi * P : (gi + 1) * P, :], in_=h3s[j][:]
                    )

    if not nc.is_finalized():
        nc.finalize()
    return nc


def kernel(
    x, edge_index, edge_attr,
    W1, b1, g1, be1, W2, b2, g2, be2, W3, b3, g3, be3,
):
    global LAST_RESULT
    W1 = np.asarray(W1, np.float32)
    W2 = np.asarray(W2, np.float32)
    W3 = np.asarray(W3, np.float32)
    # Center weights/biases over the OUTPUT feature axis so z~ = W~h + b~ is
    # exactly zero-mean across features => LayerNorm mean subtraction is free.
    W1c = W1 - W1.mean(axis=1, keepdims=True)
    W2c = W2 - W2.mean(axis=1, keepdims=True)
    W3c = W3 - W3.mean(axis=1, keepdims=True)
    b1c = np.asarray(b1, np.float32) - np.float32(np.mean(b1))
    b2c = np.asarray(b2, np.float32) - np.float32(np.mean(b2))
    b3c = np.asarray(b3, np.float32) - np.float32(np.mean(b3))

    K, per_core = _host_prep(x, edge_index, edge_attr)
    nc = _build_program(K)

    eps_col = np.full((P,), 1e-5, np.float32)
    half_col = np.full((P,), 0.5, np.float32)
    vecs = np.stack(
        [b1c, b2c, b3c]
        + [np.asarray(v, np.float32) for v in (g1, g2, g3, be1, be2, be3)]
        + [eps_col, half_col],
        axis=1,
    )  # [128, 11], column order must match VIDX in _build_program
    epick = np.zeros((P, 3), np.float32)
    epick[:, 0] = 1.0
    epick[:, 2] = 1.0
    shared = {
        "w1a": np.ascontiguousarray(W1c[:P]).astype(ml_dtypes.bfloat16),
        "w1b": np.ascontiguousarray(W1c[P:]).astype(ml_dtypes.bfloat16),
        "w2": W2c.astype(ml_dtypes.bfloat16),
        "w3": W3c.astype(ml_dtypes.bfloat16),
        "vecs": np.ascontiguousarray(vecs),
        "iota": np.ascontiguousarray(
            np.broadcast_to(np.tile(np.arange(P, dtype=np.float32), K), (P, K * P))
        ).astype(ml_dtypes.bfloat16),
        "epick": epick.astype(ml_dtypes.bfloat16),
    }
    in_maps = [
        {"edges": pay_c, "cols": col_c, "xt": xt_c, **shared}
        for (pay_c, col_c, xt_c) in per_core
    ]

    trace = bool(int(os.environ.get("KERNEL_TRACE", "0")))
    res = run_bass_kernel_spmd(nc, in_maps, core_ids=list(range(NC)), trace=trace)
    LAST_RESULT = res

    out = np.concatenate(
        [
            r["out"].reshape(G, P, GRP, P).transpose(0, 2, 3, 1).reshape(NPC, H)
            for r in res.results
        ],
        axis=0,
    )
    return np.ascontiguousarray(out[:N])
